# revision 40
# baseline (speedup 1.0000x reference)
"""Trainium2 Bass kernel for nn_DocumentLevelSelfAttention.

Math (per batch b, length L_b):
  HV = concat(H_b, V_b broadcast)              [N, 1536]
  s1 = HV @ S1_w.T                             [N, 350]
  s2 = tanh(s1) @ S2_w.T                       [N, 4]
  A  = softmax over n of masked s2             [N, 4]
  M  = A.T @ H_b                               [4, 1024]
  outputs: BM = M.flat [B, 4096], weights = A.T [B, 4, 2048]

Key restructurings:
  - cat_emb contributes a per-batch *bias* to s1 (constant over n):
    s1 = H @ S1h.T + bias_b, bias_b = S1c @ V_b  (computed on host, tiny)
  - softmax normalization done on host: device emits unnormalized masked
    exp(s2) chunks (A_un) and unnormalized M partials; host divides by
    Z = sum(A_un).  exp is safe without max-subtraction (|s2| <~ 5).
  - ragged lengths: work is chunked into 256-position chunks, only chunks
    below each batch's length are processed.  Chunks are load-balanced
    across the 8 cores at slot granularity (host pre-gathers H chunks).
  - matmuls run in float32r (fp32 with 11-bit mantissa, ~2e-4 rel err,
    4x faster than fp32 on the PE).  f32r bit encoding (round-half-even
    to 4096) is applied on the host so all loads are plain HWDGE DMAs.
  - masking is folded into the s2 PSUM as a K=1 matmul adding -1e30 at
    invalid positions, so exp() directly yields masked unnormalized A.

Device per slot (chunk): DMA H chunk [256,1024]; PE-transpose to H^T
(copies split DVE/ACT); s1^T = S1h^T.T @ H^T (K=1024); tanh(+bias) on
ACT; s2^T = S2^T.T @ tanh (K=350) + mask; exp on ACT; PE-transpose A;
M partial accumulated over the chunk's n-tiles.  The slot loop is
software-pipelined (loads 2 slots ahead, next-slot transposes emitted
between this slot's s1 and its exp-dependent tail) so the PE stays busy.
Measured ~111us per core (TimelineSim, validated against HW wall-clock
slope within a few percent).
"""

import math
import os

import numpy as np

B, N, D2, CAT, DA, R = 32, 2048, 1024, 512, 350, 4
NCORES = 8
CHUNK = int(os.environ.get("KERNEL_CHUNK", "256"))
NT = CHUNK // 128  # n-tiles per chunk
FT = D2 // 128     # f-tiles (contraction for s1)
DTS = [128, 128, 94]  # d-tile sizes (350)
FTH = int(os.environ.get("KERNEL_FTH", "0"))  # f-tiles of H^T loaded host-transposed

# "f32r" everything reduced, "mix" f32r scores + f32 M-path, "f32" exact
MODE = os.environ.get("KERNEL_MODE", "f32r")
LOOP_R = int(os.environ.get("KERNEL_LOOP_R", "0"))  # timing harness only

_COMPILED = {}
INST_LABELS = {}


def _L(bi, label):
    try:
        INST_LABELS[bi.ins.name] = label
    except Exception:
        pass
    return bi


def _build(S: int):
    import concourse.bacc as bacc
    import concourse.tile as tile
    import concourse.mybir as mybir

    f32 = mybir.dt.float32
    f32r = mybir.dt.float32r
    mm_dt = f32 if MODE == "f32" else f32r      # scores path dtype
    h_dt = f32r if MODE == "f32r" else f32      # H / M path dtype

    nc = bacc.Bacc("TRN2", target_bir_lowering=False, debug=False,
                   num_devices=NCORES)

    h_in = nc.dram_tensor("h", [S, CHUNK, D2], h_dt, kind="ExternalInput").ap()
    hth_in = (nc.dram_tensor("hth", [S, FTH, 128, CHUNK], mm_dt,
                             kind="ExternalInput").ap() if FTH else None)
    s1t_in = nc.dram_tensor("s1t", [D2, DA], mm_dt, kind="ExternalInput").ap()
    s2t_in = nc.dram_tensor("s2t", [DA, R], mm_dt, kind="ExternalInput").ap()
    bias_in = nc.dram_tensor("bias", [S, 128, 3], f32, kind="ExternalInput").ap()
    mask_in = nc.dram_tensor("mask", [S, 1, CHUNK], mm_dt, kind="ExternalInput").ap()
    ones_in = nc.dram_tensor("ones4", [1, R], mm_dt, kind="ExternalInput").ap()
    id_in = nc.dram_tensor("ident", [128, 128], f32, kind="ExternalInput").ap()
    idh_in = nc.dram_tensor("identh", [128, 128], h_dt, kind="ExternalInput").ap()
    w_out = nc.dram_tensor("wout", [S, R, CHUNK], f32, kind="ExternalOutput").ap()
    m_out = nc.dram_tensor("mout", [S, R, D2], f32, kind="ExternalOutput").ap()

    Tanh = mybir.ActivationFunctionType.Tanh
    Exp = mybir.ActivationFunctionType.Exp

    with tile.TileContext(nc) as tc:
        with tc.tile_pool(name="consts", bufs=1) as consts, \
             tc.tile_pool(name="hp", bufs=3) as hp, \
             tc.tile_pool(name="htp", bufs=3) as htp, \
             tc.tile_pool(name="slotsmall", bufs=3) as slotsmall, \
             tc.tile_pool(name="ttp", bufs=2) as ttp, \
             tc.tile_pool(name="outp", bufs=3) as outp, \
             tc.tile_pool(name="trp", bufs=3, space="PSUM") as trp, \
             tc.tile_pool(name="s1p", bufs=3, space="PSUM") as s1p, \
             tc.tile_pool(name="mp", bufs=1, space="PSUM") as mp:

            def load_slot(s):
                h = hp.tile([128, NT, D2], h_dt, tag="h")
                # per-n-tile DMAs so transposes can start earlier; slot 0
                # additionally splits nt0 by f-halves to cut warmup latency
                hsrc = h_in[s].rearrange("(nt p) f -> p nt f", p=128)
                for nt in range(NT):
                    if s == 0 and nt == 0:
                        for q in range(2):
                            nc.sync.dma_start(
                                out=h[:, 0:1, q * 512:(q + 1) * 512],
                                in_=hsrc[:, 0:1, q * 512:(q + 1) * 512])
                    else:
                        nc.sync.dma_start(
                            out=h[:, nt:nt + 1, :],
                            in_=hsrc[:, nt:nt + 1, :])
                bias_sb = slotsmall.tile([128, 3], f32, tag="bias")
                nc.sync.dma_start(out=bias_sb, in_=bias_in[s])
                mask_sb = slotsmall.tile([1, CHUNK], mm_dt, tag="mask")
                nc.sync.dma_start(out=mask_sb, in_=mask_in[s])
                ht_a = htp.tile([128, 4, CHUNK], mm_dt, tag="hta")
                ht_b = htp.tile([128, 4, CHUNK], mm_dt, tag="htb")
                if FTH:
                    nc.sync.dma_start(
                        out=ht_a[:, 0:FTH, :],
                        in_=hth_in[s].rearrange("t p n -> p t n"))
                return h, bias_sb, mask_sb, (ht_a, ht_b)

            ident_h = consts.tile([128, 128], h_dt)
            nc.sync.dma_start(out=ident_h, in_=idh_in)
            preload = None
            if not LOOP_R:
                preload = [load_slot(0)]

            s1t_sb = consts.tile([128, FT, DA], mm_dt)
            nc.sync.dma_start(
                out=s1t_sb, in_=s1t_in.rearrange("(t p) d -> p t d", p=128))
            if not LOOP_R and S > 1:
                preload.append(load_slot(1))
            s2t_sb = consts.tile([128, 3, R], mm_dt)
            for t in range(3):
                dm = DTS[t]
                nc.sync.dma_start(
                    out=s2t_sb[:dm, t, :], in_=s2t_in[t * 128:t * 128 + dm, :])
            ident = consts.tile([128, 128], f32)
            nc.sync.dma_start(out=ident, in_=id_in)
            ones4 = consts.tile([1, R], mm_dt)
            nc.sync.dma_start(out=ones4, in_=ones_in)

            def transpose_slot(h, ht_pair, fhs=(0, 1)):
                # ht half-tiles [fp, 4, n]; f-tiles < FTH arrive via DMA
                # (host-transposed, issued 2 slots ahead in load_slot)
                for fh in fhs:
                    htx = ht_pair[fh]
                    lo = max(FTH - fh * 4, 0)  # already-DMA'd tiles in group
                    if lo >= 4:
                        continue
                    g = 4 - lo
                    for nt in range(NT):
                        tp = trp.tile([128, 4, 128], h_dt, tag="tp")
                        for fi in range(g):
                            ft = fh * 4 + lo + fi
                            _L(nc.tensor.transpose(
                                tp[:, fi, :],
                                h[:, nt, ft * 128:(ft + 1) * 128],
                                ident_h), f"trH[{nt},{ft}]")
                        dst = htx[:, lo:, nt * 128:(nt + 1) * 128]
                        if fh == 0:
                            nc.vector.tensor_copy(dst, tp[:, :g, :])
                        else:
                            nc.scalar.copy(dst, tp[:, :g, :])

            def s1_slot(ht_pair, bias_sb):
                tt = ttp.tile([128, 3, CHUNK], mm_dt, tag="tt")
                for d in range(3):
                    dm = DTS[d]
                    d0 = d * 128
                    ps = s1p.tile([128, CHUNK], f32, tag="s1")
                    for ft in range(FT):
                        _L(nc.tensor.matmul(
                            ps[:dm, :],
                            lhsT=s1t_sb[:, ft, d0:d0 + dm],
                            rhs=ht_pair[ft // 4][:, ft % 4, :],
                            start=(ft == 0), stop=(ft == FT - 1)),
                           f"s1[{d},{ft}]")
                    nc.scalar.activation(tt[:dm, d, :], ps[:dm, :], Tanh,
                                         bias=bias_sb[:dm, d:d + 1])
                return tt

            def s2_slot(s, tt, mask_sb):
                # s2^T = S2^T.T @ tanh, then += (-1e30) at masked positions
                # via a K=1 matmul, so exp directly produces masked A_un
                ps2 = s1p.tile([R, CHUNK], f32, tag="s1")
                for d in range(3):
                    dm = DTS[d]
                    _L(nc.tensor.matmul(ps2,
                                     lhsT=s2t_sb[:dm, d, :],
                                     rhs=tt[:dm, d, :],
                                     start=(d == 0), stop=False), f"s2[{d}]")
                _L(nc.tensor.matmul(ps2, lhsT=ones4, rhs=mask_sb,
                                    start=False, stop=True), "s2[mask]")
                a = outp.tile([R, CHUNK], f32, tag="a")
                nc.scalar.activation(a, ps2, Exp)
                nc.sync.dma_start(out=w_out[s], in_=a)
                return a

            def tail_slot(s, h, a):
                # transpose A chunk to natural [n, r] tiles
                ep = trp.tile([128, NT, R], f32, tag="tp")
                en = slotsmall.tile([128, NT, R], h_dt, tag="en")
                for nt in range(NT):
                    _L(nc.tensor.transpose(
                        ep[:, nt, :],
                        a[:, nt * 128:(nt + 1) * 128],
                        ident[:R, :R]), f"trA[{nt}]")
                nc.vector.tensor_copy(en, ep)
                # M partial = sum_n A[n,r] H[n,f]
                pm = mp.tile([R, 2, 512], f32, tag="m")
                for nt in range(NT):
                    for fh in range(2):
                        _L(nc.tensor.matmul(
                            pm[:, fh, :],
                            lhsT=en[:, nt, :],
                            rhs=h[:, nt, fh * 512:(fh + 1) * 512],
                            start=(nt == 0), stop=(nt == NT - 1)), f"M[{nt},{fh}]")
                ms = outp.tile([R, 2, 512], f32, tag="ms")
                nc.vector.tensor_copy(ms[:, 0, :], pm[:, 0, :])
                nc.scalar.copy(ms[:, 1, :], pm[:, 1, :])
                nc.sync.dma_start(
                    out=m_out[s].rearrange("p (a b) -> p a b", a=2), in_=ms)

            # software pipeline: emission order per iteration is
            #   s1(s) | s2+exp(s) | transposes(s+1) | trA+M(s)
            # so PE filler (next-slot transposes) covers the exp latency and
            # trA+M covers part of the next slot's H^T copy latency.
            def body():
                states = preload if preload is not None else (
                    [load_slot(0)] + ([load_slot(1)] if S > 1 else []))
                transpose_slot(states[0][0], states[0][3])
                for s in range(S):
                    h, bias_sb, mask_sb, ht = states[s]
                    if s + 2 < S:
                        states.append(load_slot(s + 2))
                    tt = s1_slot(ht, bias_sb)
                    if s + 1 < S:
                        transpose_slot(states[s + 1][0], states[s + 1][3],
                                       fhs=(0,))
                    a = s2_slot(s, tt, mask_sb)
                    if s + 1 < S:
                        transpose_slot(states[s + 1][0], states[s + 1][3],
                                       fhs=(1,))
                    tail_slot(s, h, a)

            if LOOP_R:
                with tc.For_i(0, LOOP_R, 1):
                    body()
            else:
                body()

    nc.compile()
    return nc


def _get_compiled(S: int):
    key = (S, MODE)
    if key not in _COMPILED:
        _COMPILED[key] = _build(S)
    return _COMPILED[key]


def _to_f32r(x):
    b = np.ascontiguousarray(x, dtype=np.float32).view(np.uint32).astype(np.uint64)
    lsb = (b >> 12) & 1
    r = (b + 0x7FF + lsb) & ~np.uint64(0xFFF)
    return r.astype(np.uint32).view(np.float32)


def _plan(lens):
    """slot list [(b, chunk)], padded to multiple of NCORES."""
    slots = []
    for b in range(B):
        nchunks = max(1, math.ceil(min(int(lens[b]), N) / CHUNK))
        for c in range(nchunks):
            slots.append((b, c))
    S = math.ceil(len(slots) / NCORES)
    return slots, S


def _prepare(encoder_outputs, cat_emb, S1_w, S2_w, encoder_lengths):
    H = np.ascontiguousarray(np.asarray(encoder_outputs, dtype=np.float32))
    V = np.asarray(cat_emb, dtype=np.float32).reshape(B, CAT)
    S1 = np.asarray(S1_w, dtype=np.float32)
    S2 = np.asarray(S2_w, dtype=np.float32)
    lens = np.asarray(encoder_lengths).astype(np.int64).reshape(B)

    slots, S = _plan(lens)

    s1t = np.ascontiguousarray(S1[:, :D2].T)           # [1024, 350]
    s2t = np.ascontiguousarray(S2.T)                   # [350, 4]
    bias = V @ S1[:, D2:].T                            # [32, 350]
    bias_pad = np.zeros((B, 384), np.float32)
    bias_pad[:, :DA] = bias
    bias_dev = np.ascontiguousarray(
        bias_pad.reshape(B, 3, 128).transpose(0, 2, 1))  # [B, 128, 3]
    ident = np.eye(128, dtype=np.float32)
    if MODE != "f32":
        s1t_enc, s2t_enc = _to_f32r(s1t), _to_f32r(s2t)
    else:
        s1t_enc, s2t_enc = s1t, s2t
    ident_enc = _to_f32r(ident) if MODE == "f32r" else ident
    ones4_enc = np.ones((1, R), np.float32)

    in_maps = []
    slot_of_core = [[] for _ in range(NCORES)]
    for i, sl in enumerate(slots):
        slot_of_core[i % NCORES].append(sl)
    arange_chunk = np.arange(CHUNK)
    for core in range(NCORES):
        hs = np.zeros((S, CHUNK, D2), np.float32)
        bs = np.zeros((S, 128, 3), np.float32)
        ms = np.full((S, 1, CHUNK), np.float32(-1e30))
        for j, (b, c) in enumerate(slot_of_core[core]):
            seg = H[b, c * CHUNK:min((c + 1) * CHUNK, N)]
            hs[j, :seg.shape[0]] = seg
            bs[j] = bias_dev[b]
            valid = (arange_chunk + c * CHUNK < lens[b])
            ms[j, 0, valid] = 0.0
        if MODE == "f32r":
            hs = _to_f32r(hs)
        im = {"h": hs, "s1t": s1t_enc, "s2t": s2t_enc, "bias": bs,
              "mask": ms, "ident": ident, "identh": ident_enc,
              "ones4": ones4_enc}
        if FTH:
            # hth[s, t, p, n] = h[s, n, t*128+p]  (already f32r-encoded)
            im["hth"] = np.ascontiguousarray(
                hs[:, :, :FTH * 128].transpose(0, 2, 1).reshape(
                    S, FTH, 128, CHUNK))
        in_maps.append(im)
    return in_maps, slots, slot_of_core, lens, S


def kernel(encoder_outputs, cat_emb, S1_w, S2_w, encoder_lengths, batch_size,
           _return_nc=False):
    in_maps, slots, slot_of_core, lens, S = _prepare(
        encoder_outputs, cat_emb, S1_w, S2_w, encoder_lengths)
    nc = _get_compiled(S)
    if _return_nc:
        return nc, in_maps, slots, slot_of_core, lens

    from concourse.bass_utils import run_bass_kernel_spmd
    res = run_bass_kernel_spmd(nc, in_maps, list(range(NCORES)))
    return _finalize(res.results, slots, slot_of_core, lens)


def _finalize(results, slots, slot_of_core, lens):
    A_un = np.zeros((B, R, N), np.float64)
    M_un = np.zeros((B, R, D2), np.float64)
    for core in range(NCORES):
        wout = results[core]["wout"]
        mout = results[core]["mout"]
        for j, (b, c) in enumerate(slot_of_core[core]):
            hi = min((c + 1) * CHUNK, N)
            A_un[b, :, c * CHUNK:hi] = wout[j][:, :hi - c * CHUNK]
            M_un[b] += mout[j]
    Z = A_un.sum(axis=2, keepdims=True)  # [B, R, 1]
    weights = (A_un / Z).astype(np.float32)
    M = (M_un / Z).astype(np.float32)
    BM = M.reshape(B, R * D2)
    return BM, weights


# revision 41
# speedup vs baseline: 1.0115x; 1.0115x over previous
"""Trainium2 Bass kernel for nn_DocumentLevelSelfAttention.

Math (per batch b, length L_b):
  HV = concat(H_b, V_b broadcast)              [N, 1536]
  s1 = HV @ S1_w.T                             [N, 350]
  s2 = tanh(s1) @ S2_w.T                       [N, 4]
  A  = softmax over n of masked s2             [N, 4]
  M  = A.T @ H_b                               [4, 1024]
  outputs: BM = M.flat [B, 4096], weights = A.T [B, 4, 2048]

Key restructurings:
  - cat_emb contributes a per-batch *bias* to s1 (constant over n):
    s1 = H @ S1h.T + bias_b, bias_b = S1c @ V_b  (computed on host, tiny)
  - softmax normalization done on host: device emits unnormalized masked
    exp(s2) chunks (A_un) and unnormalized M partials; host divides by
    Z = sum(A_un).  exp is safe without max-subtraction (|s2| <~ 5).
  - ragged lengths: work is chunked into 256-position chunks, only chunks
    below each batch's length are processed.  Chunks are load-balanced
    across the 8 cores at slot granularity (host pre-gathers H chunks).
  - matmuls run in float32r (fp32 with 11-bit mantissa, ~2e-4 rel err,
    4x faster than fp32 on the PE).  f32r bit encoding (round-half-even
    to 4096) is applied on the host so all loads are plain HWDGE DMAs.
  - masking is folded into the s2 PSUM as a K=1 matmul adding -1e30 at
    invalid positions, so exp() directly yields masked unnormalized A.

Device per slot (chunk): DMA H chunk [256,1024]; PE-transpose to H^T
(copies split DVE/ACT); s1^T = S1h^T.T @ H^T (K=1024); tanh(+bias) on
ACT; s2^T = S2^T.T @ tanh (K=350) + mask; exp on ACT; PE-transpose A;
M partial accumulated over the chunk's n-tiles.  The slot loop is
software-pipelined (loads 2 slots ahead, next-slot transposes emitted
between this slot's s1 and its exp-dependent tail) so the PE stays busy.
Measured ~111us per core (TimelineSim, validated against HW wall-clock
slope within a few percent).
"""

import math
import os

import numpy as np

B, N, D2, CAT, DA, R = 32, 2048, 1024, 512, 350, 4
NCORES = 8
CHUNK = int(os.environ.get("KERNEL_CHUNK", "256"))
NT = CHUNK // 128  # n-tiles per chunk
FT = D2 // 128     # f-tiles (contraction for s1)
DTS = [128, 128, 94]  # d-tile sizes (350)
FTH = int(os.environ.get("KERNEL_FTH", "0"))  # f-tiles of H^T loaded host-transposed

# "f32r" everything reduced, "mix" f32r scores + f32 M-path, "f32" exact
MODE = os.environ.get("KERNEL_MODE", "f32r")
LOOP_R = int(os.environ.get("KERNEL_LOOP_R", "0"))  # timing harness only

_COMPILED = {}
INST_LABELS = {}


def _L(bi, label):
    try:
        INST_LABELS[bi.ins.name] = label
    except Exception:
        pass
    return bi


def _build(S: int):
    import concourse.bacc as bacc
    import concourse.tile as tile
    import concourse.mybir as mybir

    f32 = mybir.dt.float32
    f32r = mybir.dt.float32r
    mm_dt = f32 if MODE == "f32" else f32r      # scores path dtype
    h_dt = f32r if MODE == "f32r" else f32      # H / M path dtype

    nc = bacc.Bacc("TRN2", target_bir_lowering=False, debug=False,
                   num_devices=NCORES)

    h_in = nc.dram_tensor("h", [S, CHUNK, D2], h_dt, kind="ExternalInput").ap()
    hth_in = (nc.dram_tensor("hth", [S, FTH, 128, CHUNK], mm_dt,
                             kind="ExternalInput").ap() if FTH else None)
    s1t_in = nc.dram_tensor("s1t", [D2, DA], mm_dt, kind="ExternalInput").ap()
    s2t_in = nc.dram_tensor("s2t", [DA, R], mm_dt, kind="ExternalInput").ap()
    bias_in = nc.dram_tensor("bias", [S, 128, 3], f32, kind="ExternalInput").ap()
    mask_in = nc.dram_tensor("mask", [S, 1, CHUNK], mm_dt, kind="ExternalInput").ap()
    ones_in = nc.dram_tensor("ones4", [1, R], mm_dt, kind="ExternalInput").ap()
    id_in = nc.dram_tensor("ident", [128, 128], f32, kind="ExternalInput").ap()
    idh_in = nc.dram_tensor("identh", [128, 128], h_dt, kind="ExternalInput").ap()
    w_out = nc.dram_tensor("wout", [S, R, CHUNK], f32, kind="ExternalOutput").ap()
    m_out = nc.dram_tensor("mout", [S, R, D2], f32, kind="ExternalOutput").ap()

    Tanh = mybir.ActivationFunctionType.Tanh
    Exp = mybir.ActivationFunctionType.Exp

    with tile.TileContext(nc) as tc:
        with tc.tile_pool(name="consts", bufs=1) as consts, \
             tc.tile_pool(name="hp", bufs=3) as hp, \
             tc.tile_pool(name="htp", bufs=3) as htp, \
             tc.tile_pool(name="slotsmall", bufs=3) as slotsmall, \
             tc.tile_pool(name="ttp", bufs=2) as ttp, \
             tc.tile_pool(name="outp", bufs=3) as outp, \
             tc.tile_pool(name="trp", bufs=4, space="PSUM") as trp, \
             tc.tile_pool(name="s1p", bufs=2, space="PSUM") as s1p, \
             tc.tile_pool(name="mp", bufs=1, space="PSUM") as mp:

            def load_slot(s):
                h = hp.tile([128, NT, D2], h_dt, tag="h")
                # per-n-tile DMAs so transposes can start earlier; slot 0
                # additionally splits nt0 by f-halves to cut warmup latency
                hsrc = h_in[s].rearrange("(nt p) f -> p nt f", p=128)
                for nt in range(NT):
                    if s == 0 and nt == 0:
                        for q in range(2):
                            nc.sync.dma_start(
                                out=h[:, 0:1, q * 512:(q + 1) * 512],
                                in_=hsrc[:, 0:1, q * 512:(q + 1) * 512])
                    else:
                        nc.sync.dma_start(
                            out=h[:, nt:nt + 1, :],
                            in_=hsrc[:, nt:nt + 1, :])
                bias_sb = slotsmall.tile([128, 3], f32, tag="bias")
                nc.sync.dma_start(out=bias_sb, in_=bias_in[s])
                mask_sb = slotsmall.tile([1, CHUNK], mm_dt, tag="mask")
                nc.sync.dma_start(out=mask_sb, in_=mask_in[s])
                ht_a = htp.tile([128, 4, CHUNK], mm_dt, tag="hta")
                ht_b = htp.tile([128, 4, CHUNK], mm_dt, tag="htb")
                if FTH:
                    nc.sync.dma_start(
                        out=ht_a[:, 0:FTH, :],
                        in_=hth_in[s].rearrange("t p n -> p t n"))
                return h, bias_sb, mask_sb, (ht_a, ht_b)

            ident_h = consts.tile([128, 128], h_dt)
            nc.sync.dma_start(out=ident_h, in_=idh_in)
            preload = None
            if not LOOP_R:
                preload = [load_slot(0)]

            s1t_sb = consts.tile([128, FT, DA], mm_dt)
            nc.sync.dma_start(
                out=s1t_sb, in_=s1t_in.rearrange("(t p) d -> p t d", p=128))
            if not LOOP_R and S > 1:
                preload.append(load_slot(1))
            s2t_sb = consts.tile([128, 3, R], mm_dt)
            for t in range(3):
                dm = DTS[t]
                nc.sync.dma_start(
                    out=s2t_sb[:dm, t, :], in_=s2t_in[t * 128:t * 128 + dm, :])
            ident = consts.tile([128, 128], f32)
            nc.sync.dma_start(out=ident, in_=id_in)
            ones4 = consts.tile([1, R], mm_dt)
            nc.sync.dma_start(out=ones4, in_=ones_in)

            def transpose_slot(h, ht_pair, fhs=(0, 1)):
                # ht half-tiles [fp, 4, n]; f-tiles < FTH arrive via DMA
                # (host-transposed, issued 2 slots ahead in load_slot)
                for fh in fhs:
                    htx = ht_pair[fh]
                    lo = max(FTH - fh * 4, 0)  # already-DMA'd tiles in group
                    if lo >= 4:
                        continue
                    g = 4 - lo
                    for nt in range(NT):
                        tp = trp.tile([128, 4, 128], h_dt, tag="tp")
                        for fi in range(g):
                            ft = fh * 4 + lo + fi
                            _L(nc.tensor.transpose(
                                tp[:, fi, :],
                                h[:, nt, ft * 128:(ft + 1) * 128],
                                ident_h), f"trH[{nt},{ft}]")
                        dst = htx[:, lo:, nt * 128:(nt + 1) * 128]
                        if fh == 0:
                            nc.vector.tensor_copy(dst, tp[:, :g, :])
                        else:
                            nc.scalar.copy(dst, tp[:, :g, :])

            def s1_slot(ht_pair, bias_sb):
                tt = ttp.tile([128, 3, CHUNK], mm_dt, tag="tt")
                for d in range(3):
                    dm = DTS[d]
                    d0 = d * 128
                    ps = s1p.tile([128, CHUNK], f32, tag="s1")
                    for ft in range(FT):
                        _L(nc.tensor.matmul(
                            ps[:dm, :],
                            lhsT=s1t_sb[:, ft, d0:d0 + dm],
                            rhs=ht_pair[ft // 4][:, ft % 4, :],
                            start=(ft == 0), stop=(ft == FT - 1)),
                           f"s1[{d},{ft}]")
                    nc.scalar.activation(tt[:dm, d, :], ps[:dm, :], Tanh,
                                         bias=bias_sb[:dm, d:d + 1])
                return tt

            def s2_slot(s, tt, mask_sb):
                # s2^T = S2^T.T @ tanh, then += (-1e30) at masked positions
                # via a K=1 matmul, so exp directly produces masked A_un
                ps2 = s1p.tile([R, CHUNK], f32, tag="s1")
                for d in range(3):
                    dm = DTS[d]
                    _L(nc.tensor.matmul(ps2,
                                     lhsT=s2t_sb[:dm, d, :],
                                     rhs=tt[:dm, d, :],
                                     start=(d == 0), stop=False), f"s2[{d}]")
                _L(nc.tensor.matmul(ps2, lhsT=ones4, rhs=mask_sb,
                                    start=False, stop=True), "s2[mask]")
                a = outp.tile([R, CHUNK], f32, tag="a")
                nc.scalar.activation(a, ps2, Exp)
                nc.sync.dma_start(out=w_out[s], in_=a)
                return a

            def tail_slot(s, h, a):
                # transpose A chunk to natural [n, r] tiles
                ep = trp.tile([128, NT, R], f32, tag="tp")
                en = slotsmall.tile([128, NT, R], h_dt, tag="en")
                for nt in range(NT):
                    _L(nc.tensor.transpose(
                        ep[:, nt, :],
                        a[:, nt * 128:(nt + 1) * 128],
                        ident[:R, :R]), f"trA[{nt}]")
                nc.vector.tensor_copy(en, ep)
                # M partial = sum_n A[n,r] H[n,f]
                pm = mp.tile([R, 2, 512], f32, tag="m")
                for nt in range(NT):
                    for fh in range(2):
                        _L(nc.tensor.matmul(
                            pm[:, fh, :],
                            lhsT=en[:, nt, :],
                            rhs=h[:, nt, fh * 512:(fh + 1) * 512],
                            start=(nt == 0), stop=(nt == NT - 1)), f"M[{nt},{fh}]")
                ms = outp.tile([R, 2, 512], f32, tag="ms")
                nc.vector.tensor_copy(ms[:, 0, :], pm[:, 0, :])
                nc.scalar.copy(ms[:, 1, :], pm[:, 1, :])
                nc.sync.dma_start(
                    out=m_out[s].rearrange("p (a b) -> p a b", a=2), in_=ms)

            # software pipeline: emission order per iteration is
            #   s1(s) | s2+exp(s) | transposes(s+1) | trA+M(s)
            # so PE filler (next-slot transposes) covers the exp latency and
            # trA+M covers part of the next slot's H^T copy latency.
            def body():
                states = preload if preload is not None else (
                    [load_slot(0)] + ([load_slot(1)] if S > 1 else []))
                transpose_slot(states[0][0], states[0][3])
                for s in range(S):
                    h, bias_sb, mask_sb, ht = states[s]
                    if s + 2 < S:
                        states.append(load_slot(s + 2))
                    tt = s1_slot(ht, bias_sb)
                    if s + 1 < S:
                        transpose_slot(states[s + 1][0], states[s + 1][3],
                                       fhs=(0,))
                    a = s2_slot(s, tt, mask_sb)
                    if s + 1 < S:
                        transpose_slot(states[s + 1][0], states[s + 1][3],
                                       fhs=(1,))
                    tail_slot(s, h, a)

            if LOOP_R:
                with tc.For_i(0, LOOP_R, 1):
                    body()
            else:
                body()

    nc.compile()
    return nc


def _get_compiled(S: int):
    key = (S, MODE)
    if key not in _COMPILED:
        _COMPILED[key] = _build(S)
    return _COMPILED[key]


def _to_f32r(x):
    b = np.ascontiguousarray(x, dtype=np.float32).view(np.uint32).astype(np.uint64)
    lsb = (b >> 12) & 1
    r = (b + 0x7FF + lsb) & ~np.uint64(0xFFF)
    return r.astype(np.uint32).view(np.float32)


def _plan(lens):
    """slot list [(b, chunk)], padded to multiple of NCORES."""
    slots = []
    for b in range(B):
        nchunks = max(1, math.ceil(min(int(lens[b]), N) / CHUNK))
        for c in range(nchunks):
            slots.append((b, c))
    S = math.ceil(len(slots) / NCORES)
    return slots, S


def _prepare(encoder_outputs, cat_emb, S1_w, S2_w, encoder_lengths):
    H = np.ascontiguousarray(np.asarray(encoder_outputs, dtype=np.float32))
    V = np.asarray(cat_emb, dtype=np.float32).reshape(B, CAT)
    S1 = np.asarray(S1_w, dtype=np.float32)
    S2 = np.asarray(S2_w, dtype=np.float32)
    lens = np.asarray(encoder_lengths).astype(np.int64).reshape(B)

    slots, S = _plan(lens)

    s1t = np.ascontiguousarray(S1[:, :D2].T)           # [1024, 350]
    s2t = np.ascontiguousarray(S2.T)                   # [350, 4]
    bias = V @ S1[:, D2:].T                            # [32, 350]
    bias_pad = np.zeros((B, 384), np.float32)
    bias_pad[:, :DA] = bias
    bias_dev = np.ascontiguousarray(
        bias_pad.reshape(B, 3, 128).transpose(0, 2, 1))  # [B, 128, 3]
    ident = np.eye(128, dtype=np.float32)
    if MODE != "f32":
        s1t_enc, s2t_enc = _to_f32r(s1t), _to_f32r(s2t)
    else:
        s1t_enc, s2t_enc = s1t, s2t
    ident_enc = _to_f32r(ident) if MODE == "f32r" else ident
    ones4_enc = np.ones((1, R), np.float32)

    in_maps = []
    slot_of_core = [[] for _ in range(NCORES)]
    for i, sl in enumerate(slots):
        slot_of_core[i % NCORES].append(sl)
    arange_chunk = np.arange(CHUNK)
    for core in range(NCORES):
        hs = np.zeros((S, CHUNK, D2), np.float32)
        bs = np.zeros((S, 128, 3), np.float32)
        ms = np.full((S, 1, CHUNK), np.float32(-1e30))
        for j, (b, c) in enumerate(slot_of_core[core]):
            seg = H[b, c * CHUNK:min((c + 1) * CHUNK, N)]
            hs[j, :seg.shape[0]] = seg
            bs[j] = bias_dev[b]
            valid = (arange_chunk + c * CHUNK < lens[b])
            ms[j, 0, valid] = 0.0
        if MODE == "f32r":
            hs = _to_f32r(hs)
        im = {"h": hs, "s1t": s1t_enc, "s2t": s2t_enc, "bias": bs,
              "mask": ms, "ident": ident, "identh": ident_enc,
              "ones4": ones4_enc}
        if FTH:
            # hth[s, t, p, n] = h[s, n, t*128+p]  (already f32r-encoded)
            im["hth"] = np.ascontiguousarray(
                hs[:, :, :FTH * 128].transpose(0, 2, 1).reshape(
                    S, FTH, 128, CHUNK))
        in_maps.append(im)
    return in_maps, slots, slot_of_core, lens, S


def kernel(encoder_outputs, cat_emb, S1_w, S2_w, encoder_lengths, batch_size,
           _return_nc=False):
    in_maps, slots, slot_of_core, lens, S = _prepare(
        encoder_outputs, cat_emb, S1_w, S2_w, encoder_lengths)
    nc = _get_compiled(S)
    if _return_nc:
        return nc, in_maps, slots, slot_of_core, lens

    from concourse.bass_utils import run_bass_kernel_spmd
    res = run_bass_kernel_spmd(nc, in_maps, list(range(NCORES)))
    return _finalize(res.results, slots, slot_of_core, lens)


def _finalize(results, slots, slot_of_core, lens):
    A_un = np.zeros((B, R, N), np.float64)
    M_un = np.zeros((B, R, D2), np.float64)
    for core in range(NCORES):
        wout = results[core]["wout"]
        mout = results[core]["mout"]
        for j, (b, c) in enumerate(slot_of_core[core]):
            hi = min((c + 1) * CHUNK, N)
            A_un[b, :, c * CHUNK:hi] = wout[j][:, :hi - c * CHUNK]
            M_un[b] += mout[j]
    Z = A_un.sum(axis=2, keepdims=True)  # [B, R, 1]
    weights = (A_un / Z).astype(np.float32)
    M = (M_un / Z).astype(np.float32)
    BM = M.reshape(B, R * D2)
    return BM, weights
